# revision 8
# baseline (speedup 1.0000x reference)
"""MeshUnPool gather kernel for 8 Trainium2 NeuronCores.

reference: out[i, :] = features[parent_idx[i], :]
  features: [500000, 256] f32 (512 MB), parent_idx: [1000000] int64/int32,
  out: [1000000, 256] f32 (1 GB).

Sharding: output rows sharded across the 8 cores; feature table replicated.

Per core (125952 rows): indirect row-gather DMAs (128 rows/instruction --
the SWDGE ucode consumes one int32 index per SBUF partition) pull random
1KB table rows into SBUF. Rows are assigned p-major (gather (b,j) covers
rows b*3072 + p*24 + j), so each partition's 24 rows per block are
CONTIGUOUS in the output and the store flushes as fat 24KB descriptors,
alternating across both HWDGE queues (sync/scalar). The baseline wrote
1KB store descriptors through one HWDGE queue (~10.7 ns each, 1.35 ms);
fat descriptors collapse the store side so the wall is the GPSIMD
desc-gen floor (984 instructions x ~1.09 us). Output is bit-exact.
"""

import numpy as np

import concourse.bass as bass
import concourse.bacc as bacc
import concourse.mybir as mybir
import concourse.tile as tile
from concourse.bass_utils import run_bass_kernel_spmd

N_POOLED = 500000
N_UNPOOLED = 1000000
C = 256
NCORES = 8
P = 128

# rows per core = P * GPB * NB ; 8 * 125952 = 1007616 (0.76% pad)
GPB = 24          # gathers (128 rows each) per store block
NB = 41           # store blocks per core
ROWS_PER_CORE = P * GPB * NB

_cache = {}


def _emit_indirect(nc, out, in_, idx_ap, queue_name):
    """indirect_dma_start clone with a selectable SWDGE queue name."""
    eng = nc.gpsimd
    out_ap = eng.lower_ap_dma(out, for_indirect_dma=True)
    in_ap = eng.lower_ap_dma(in_, for_indirect_dma=True)
    assert len(in_ap) == 1 and len(out_ap) == 1
    offset_ap = eng.lower_ap_dma(idx_ap)
    assert len(offset_ap) == 1
    in_ap.append(offset_ap[0])
    coef = 1
    for d in in_.shape[1:]:
        coef *= d
    in_ap[0].dynamic_ap_info = mybir.DynamicAccessPatternInfo(
        c=0,
        actual_ap=out.ap,
        indirect_dim_max_index=in_.shape[0],
        offset_expr=[mybir.DynamicAccessPatternOffsetExpr(
            coef=coef,
            aff_expr=mybir.DynamicAccessPatternOffsetExprAffExpr(
                kind="IndirectArgId", arg_id=1),
        )],
    )
    return eng.add_instruction(mybir.InstDMACopy(
        name=nc.get_next_instruction_name(),
        queue=queue_name,
        mode="Copy",
        ins=in_ap,
        outs=out_ap,
        oob_is_err=True,
        cce_op=mybir.AluOpType.bypass,
    ))


def _align_queues(nc):
    """SWDGE sem lanes are locked one queue each; rename every indirect
    DMA's queue to follow its Tile-assigned DMASW lane (queue := lane%4)."""
    DMASW0 = 11
    n = 0
    for inst in nc.inst_map.values():
        if isinstance(inst, mybir.InstDMACopy) and \
                str(inst.queue).startswith("qPoolDynamic"):
            proc = inst.bass_scheduled_proc
            assert proc is not None and DMASW0 <= proc < DMASW0 + 8, (
                f"{inst.name}: proc={proc}")
            lane = (proc - DMASW0) % 4
            inst.queue = f"qPoolDynamic{lane if lane else ''}"
            n += 1
    assert n > 0


def _build():
    nc = bacc.Bacc("TRN2", target_bir_lowering=False, debug=False,
                   num_devices=NCORES, num_swdge_queues=4)
    feat = nc.dram_tensor("features", [N_POOLED, C], mybir.dt.float32,
                          kind="ExternalInput").ap()
    # host ships idx p-major: element (p, b*GPB+j) = idx[b*3072 + p*GPB + j]
    idx = nc.dram_tensor("parent_idx", [P, GPB * NB], mybir.dt.int32,
                         kind="ExternalInput").ap()
    out = nc.dram_tensor("out", [ROWS_PER_CORE, C], mybir.dt.float32,
                         kind="ExternalOutput").ap()

    with tile.TileContext(nc) as tc:
        with tc.tile_pool(name="g", bufs=3) as gp, \
             tc.tile_pool(name="i", bufs=1) as ip:
            idx_tile = ip.tile([P, GPB * NB], mybir.dt.int32)
            nc.scalar.dma_start(out=idx_tile[:], in_=idx[:])
            for b in range(NB):
                gtile = gp.tile([P, GPB * C], mybir.dt.float32)
                for j in range(GPB):
                    t = b * GPB + j
                    _emit_indirect(nc, gtile[:, j * C:(j + 1) * C], feat[:],
                                   idx_tile[:, t:t + 1],
                                   f"qPoolDynamic{t % 4 if t % 4 else ''}")
                # block rows p-major: row b*3072 + p*GPB + j = gtile[p, j]
                eng = nc.sync if b % 2 == 0 else nc.scalar
                eng.dma_start(
                    out=out[b * GPB * P:(b + 1) * GPB * P, :].rearrange(
                        "(p j) c -> p j c", p=P),
                    in_=gtile[:].rearrange("p (j c) -> p j c", c=C),
                )
    _align_queues(nc)
    nc.compile()
    return nc


def _run(features, parent_idx, **spmd_kwargs):
    feat = np.ascontiguousarray(np.asarray(features), dtype=np.float32)
    idx32 = np.zeros(ROWS_PER_CORE * NCORES, dtype=np.int32)
    idx32[:N_UNPOOLED] = np.asarray(parent_idx).astype(np.int32)
    # per core: row b*128*GPB + p*GPB + j  ->  idx element (p, b*GPB + j)
    shards = (idx32.reshape(NCORES, NB, P, GPB)
              .transpose(0, 2, 1, 3).reshape(NCORES, P, NB * GPB))

    if "nc" not in _cache:
        _cache["nc"] = _build()
    nc = _cache["nc"]

    in_maps = [{"features": feat,
                "parent_idx": np.ascontiguousarray(shards[c])}
               for c in range(NCORES)]
    res = run_bass_kernel_spmd(nc, in_maps, core_ids=list(range(NCORES)),
                               **spmd_kwargs)
    out = np.concatenate([r["out"] for r in res.results], axis=0)[:N_UNPOOLED]
    return out, res


def kernel(features, parent_idx):
    out, _ = _run(features, parent_idx)
    return out
